# revision 1
# baseline (speedup 1.0000x reference)
"""BitLinear kernel for Trainium2, 8 NeuronCores, column-parallel.

y[t, o] = sum_i x[t, i] * sign(W[o, i]) * scale[o]
  x: [8192, 4096] f32 (replicated), W: [16384, 4096] f32, scale: [16384] f32
  Each core owns OUT_F/8 = 2048 output features (column parallel).

Per-core pipeline (all math on device):
  - W prep:   W f32 --SWDGE casting DMA--> f16 SBUF; sign(w)*scale via one
              DVE bit op on u16 views: (w16 & 0x8000) ^ bits(f16(scale[o]))
              (exact: w has no exact zeros and the trick only uses w's sign
              bit); PE-transpose (grouped 4 per PSUM bank) -> B [128,32,2048]
              f16 resident.  Emitted band-by-band (512 outputs per band).
  - x path:   x f32 --casting DMA--> xc f16 [128, 4096]; PE-transpose ->
              xT [128, 32, 128] f16 (ScalarE copies PSUM->SBUF).
  - matmul:   per 128-token tile, 4 x (32 fp16 matmuls accumulating K into
              PSUM [128, 512] f32) -> DVE copy -> y quarter -> DMA out.
The first WARM token tiles issue band-major right behind W prep so the
PE never waits on weight preparation.  fp16 keeps sign*scale exact; only x
quantizes (~2e-4 rel err); PSUM accumulates in f32.
"""

import os
import sys

for _p in ("/opt/trn_rl_repo",):
    if _p not in sys.path and os.path.isdir(_p):
        sys.path.append(_p)

import numpy as np
import concourse.bacc as bacc
import concourse.mybir as mybir
from concourse.tile import TileContext
from concourse.masks import make_identity
from concourse.bass_utils import run_bass_kernel_spmd

TOKENS, IN_F, OUT_F, NCORES = 8192, 4096, 16384, 8
O_SH = OUT_F // NCORES  # 2048 out features per core
P = 128
KT = IN_F // P          # 32 k-subtiles
MT = TOKENS // P        # 64 token tiles
OT = O_SH // P          # 16 o-tiles per core
W_KC = 2048             # W prep free-dim chunk
NBAND = 4               # 4 output bands of 512
WARM = 5                # band-major warm token tiles

f32, f16, u16 = mybir.dt.float32, mybir.dt.float16, mybir.dt.uint16
AF = mybir.ActivationFunctionType

_CACHE = {}
last_result = None


def build():
    nc = bacc.Bacc("TRN2", target_bir_lowering=False, debug=False)
    x = nc.dram_tensor("x", [TOKENS, IN_F], f32, kind="ExternalInput").ap()
    w = nc.dram_tensor("weight", [O_SH, IN_F], f32, kind="ExternalInput").ap()
    scale = nc.dram_tensor("scale", [O_SH], f32, kind="ExternalInput").ap()
    y = nc.dram_tensor("y", [TOKENS, O_SH], f32, kind="ExternalOutput").ap()

    with TileContext(nc) as tc:
        with (
            tc.tile_pool(name="const", bufs=1) as cpool,
            tc.tile_pool(name="bres", bufs=1) as bpool,
            tc.tile_pool(name="wstage", bufs=3) as wpool,
            tc.tile_pool(name="xstage", bufs=2) as xpool,
            tc.tile_pool(name="xtp", bufs=WARM) as xtpool,
            tc.tile_pool(name="ystage", bufs=4) as ypool,
            tc.tile_pool(name="mmps", bufs=5, space="PSUM") as mmps,
            tc.tile_pool(name="tpps", bufs=3, space="PSUM") as tpps,
        ):
            ident = cpool.tile([P, P], f16, tag="ident")
            make_identity(nc, ident)
            scale_sb = cpool.tile([P, OT], f32, tag="scale")
            nc.sync.dma_start(scale_sb[:], scale.rearrange("(o p) -> p o", p=P))
            scale16 = cpool.tile([P, OT], f16, tag="scale16")
            nc.vector.tensor_copy(scale16[:], scale_sb[:])

            B = bpool.tile([P, KT, O_SH], f16, tag="B")

            def prep_w_band(band):
                """Produce B[:, :, band*512:(band+1)*512]."""
                copy_flip = band % 2
                for oi in range(4):
                    ot = band * 4 + oi
                    for kc in range(IN_F // W_KC):  # chunks of W_KC
                        wsg = wpool.tile([P, W_KC], f16, tag="wsg")
                        # f32 -> f16 during the DMA itself (SWDGE cast)
                        nc.gpsimd.dma_start(
                            wsg[:],
                            w[ot * P : (ot + 1) * P, kc * W_KC : (kc + 1) * W_KC],
                        )
                        # sign(w)*scale = (w16 & 0x8000) ^ bits(f16(scale[o]))
                        nc.vector.tensor_scalar(
                            wsg[:].bitcast(u16),
                            wsg[:].bitcast(u16),
                            0x8000,
                            scale16[:, ot : ot + 1].bitcast(u16),
                            mybir.AluOpType.bitwise_and,
                            mybir.AluOpType.bitwise_xor,
                        )
                        ksub0 = kc * (W_KC // P)
                        for g in range(W_KC // P // 4):  # groups of 4
                            tp = tpps.tile([P, 512], f16, tag="tp")
                            for j in range(4):
                                ki = g * 4 + j
                                nc.tensor.transpose(
                                    tp[:, j * P : (j + 1) * P],
                                    wsg[:, ki * P : (ki + 1) * P],
                                    ident[:],
                                )
                            k0 = ksub0 + g * 4
                            dst = B[:, k0 : k0 + 4, ot * P : (ot + 1) * P]
                            src = tp[:].rearrange("p (a b) -> p a b", a=4)
                            # alternate copy engine to balance ACT/DVE
                            if (g + oi + copy_flip) % 2 == 0:
                                nc.vector.tensor_copy(dst, src)
                            else:
                                nc.scalar.activation(dst, src, AF.Copy)

            def make_xT(mt):
                xc = xpool.tile([P, IN_F], f16, tag="xc")
                nc.gpsimd.dma_start(xc[:], x[mt * P : (mt + 1) * P, :])
                xT = xtpool.tile([P, KT, P], f16, tag="xT")
                for g in range(KT // 4):  # 8 groups of 4 transposes
                    tp = tpps.tile([P, 512], f16, tag="tp")
                    for j in range(4):
                        ki = g * 4 + j
                        nc.tensor.transpose(
                            tp[:, j * P : (j + 1) * P],
                            xc[:, ki * P : (ki + 1) * P],
                            ident[:],
                        )
                    nc.scalar.activation(
                        xT[:, g * 4 : g * 4 + 4, :],
                        tp[:].rearrange("p (a b) -> p a b", a=4),
                        AF.Copy,
                    )
                return xT

            def mm_band(mt, band, xT):
                ps = mmps.tile([P, 512], f32, tag="ps")
                n0 = band * 512
                for k in range(KT):
                    nc.tensor.matmul(
                        ps[:],
                        xT[:, k, :],
                        B[:, k, n0 : n0 + 512],
                        start=(k == 0),
                        stop=(k == KT - 1),
                    )
                yq = ypool.tile([P, 512], f32, tag="yq")
                nc.vector.tensor_copy(yq[:], ps[:])
                nc.sync.dma_start(
                    y[mt * P : (mt + 1) * P, n0 : n0 + 512], yq[:]
                )

            # W prep band 0, warm xT tiles, then band-major warm MMs with
            # each next band's prep emitted BEFORE the current band's MMs
            # (keeps prep ops ahead of MM-gated y-copies in the engine FIFOs)
            prep_w_band(0)
            warm_xT = [make_xT(mt) for mt in range(WARM)]
            for band in range(NBAND):
                if band + 1 < NBAND:
                    prep_w_band(band + 1)
                for mt in range(WARM):
                    mm_band(mt, band, warm_xT[mt])

            # steady phase
            for mt in range(WARM, MT):
                xT = make_xT(mt)
                for band in range(NBAND):
                    mm_band(mt, band, xT)

    nc.finalize()
    return nc


def _get_nc():
    if "nc" not in _CACHE:
        _CACHE["nc"] = build()
    return _CACHE["nc"]


def kernel(x, weight, scale):
    global last_result
    nc = _get_nc()
    x = np.ascontiguousarray(np.asarray(x, dtype=np.float32))
    weight = np.ascontiguousarray(np.asarray(weight, dtype=np.float32))
    scale = np.ascontiguousarray(np.asarray(scale, dtype=np.float32))
    in_maps = [
        {
            "x": x,
            "weight": np.ascontiguousarray(weight[c * O_SH : (c + 1) * O_SH]),
            "scale": np.ascontiguousarray(scale[c * O_SH : (c + 1) * O_SH]),
        }
        for c in range(NCORES)
    ]
    res = run_bass_kernel_spmd(nc, in_maps, list(range(NCORES)))
    last_result = res
    return np.concatenate([res.results[c]["y"] for c in range(NCORES)], axis=1)


if __name__ == "__main__":
    rng = np.random.default_rng(0)
    xv = rng.standard_normal((TOKENS, IN_F), dtype=np.float32)
    wv = rng.standard_normal((OUT_F, IN_F), dtype=np.float32)
    sv = np.ones(OUT_F, dtype=np.float32)
    yv = kernel(xv, wv, sv)
    print("out shape:", yv.shape, yv.dtype)



# revision 2
# speedup vs baseline: 1.2585x; 1.2585x over previous
"""BitLinear kernel for Trainium2, 8 NeuronCores, column-parallel.

y[t, o] = sum_i x[t, i] * sign(W[o, i]) * scale[o]
  x: [8192, 4096] f32 (replicated), W: [16384, 4096] f32, scale: [16384] f32
  Each core owns OUT_F/8 = 2048 output features (column parallel).

Mixed-precision contraction (per-core):
  - k in [0, 2048):    fp16 MMs (K=128 each, 16 per band)
  - k in [2048, 4096): fp8e4 DoubleRow MMs (K=256 each, 8 per band)
    DR runs at 2x: K=256 per ~216ns vs K=128 for fp16 — verified on HW.
  Quantizing half of x to e4m3 adds ~1.9e-2 relative output error
  (BitLinear sign weights are exact +-1 in both f16 and fp8).

Per-core pipeline (all math on device):
  - W prep:   W f32 --casting DMA--> f16; sign via bit trick
              (w16 & 0x8000) ^ 0x3C00 -> +-1.0 f16 exactly; PE-transpose;
              PSUM copies write B16 f16 (k<2048) and B8 fp8e4 (k>=2048).
  - scale:    SCB [128, 2048] f32 broadcast tile (scale replicated across
              token partitions) via per-partition DMA; output stage
              multiplies PSUM by SCB slice (general scale support).
  - x path:   x f32 --casting DMA--> f16 [128, 4096]; PE-transpose;
              PSUM copies write xT f16 (k<2048) and xT8 fp8e4 (rest).
  - matmul:   per 128-token tile: k-outer/band-inner: 16 f16 MMs x 4 bands
              accumulate PSUM, then 8 DR MMs x 4 bands finish; DVE
              multiplies PSUM by SCB -> y tile -> DMA out.
"""

import os
import sys

for _p in ("/opt/trn_rl_repo",):
    if _p not in sys.path and os.path.isdir(_p):
        sys.path.append(_p)

import numpy as np
import concourse.bacc as bacc
import concourse.mybir as mybir
from concourse.tile import TileContext
from concourse.masks import make_identity
from concourse.bass_utils import run_bass_kernel_spmd

TOKENS, IN_F, OUT_F, NCORES = 8192, 4096, 16384, 8
O_SH = OUT_F // NCORES  # 2048 out features per core
P = 128
KT = IN_F // P          # 32 k-subtiles total
KT16 = 16               # k-subtiles done in f16 (k < 2048)
KT8 = KT - KT16         # k-subtiles done in fp8 DR (16 -> 8 DR MMs)
MT = TOKENS // P        # 64 token tiles
NBAND = 4               # 4 output bands of 512
NB = 512
W_KC = 2048             # W prep free-dim chunk (= half the k range)
WARM = 5                # band-major warm token tiles

f32, f16, u16 = mybir.dt.float32, mybir.dt.float16, mybir.dt.uint16
fp8 = mybir.dt.float8e4
DR = mybir.MatmulPerfMode.DoubleRow
AF = mybir.ActivationFunctionType

_CACHE = {}
last_result = None


def build():
    nc = bacc.Bacc("TRN2", target_bir_lowering=False, debug=False)
    x = nc.dram_tensor("x", [TOKENS, IN_F], f32, kind="ExternalInput").ap()
    w = nc.dram_tensor("weight", [O_SH, IN_F], f32, kind="ExternalInput").ap()
    scale = nc.dram_tensor("scale", [O_SH], f32, kind="ExternalInput").ap()
    y = nc.dram_tensor("y", [TOKENS, O_SH], f32, kind="ExternalOutput").ap()

    with TileContext(nc) as tc:
        with (
            tc.tile_pool(name="const", bufs=1) as cpool,
            tc.tile_pool(name="bres", bufs=1) as bpool,
            tc.tile_pool(name="wstage", bufs=3) as wpool,
            tc.tile_pool(name="xstage", bufs=2) as xpool,
            tc.tile_pool(name="xtp", bufs=WARM) as xtpool,
            tc.tile_pool(name="ystage", bufs=4) as ypool,
            tc.tile_pool(name="mmps", bufs=4, space="PSUM") as mmps,
            tc.tile_pool(name="tpps", bufs=3, space="PSUM") as tpps,
        ):
            ident = cpool.tile([P, P], f16, tag="ident")
            make_identity(nc, ident)

            # broadcast scale tile: SCB[p, o] = scale[o] for all p
            SCB = cpool.tile([P, O_SH], f32, tag="scb")
            for p in range(P):
                nc.sync.dma_start(SCB[p : p + 1, :], scale)

            B16 = bpool.tile([P, KT16, O_SH], f16, tag="B16")
            B8 = bpool.tile([P, KT8, O_SH], fp8, tag="B8")

            def prep_w_band(band):
                """Produce B16/B8 [:, :, band*512:(band+1)*512]."""
                copy_flip = band % 2
                for oi in range(4):
                    ot = band * 4 + oi
                    for kc in range(IN_F // W_KC):  # kc=0 -> f16, kc=1 -> fp8
                        wsg = wpool.tile([P, W_KC], f16, tag="wsg")
                        # f32 -> f16 during the DMA itself (SWDGE cast)
                        nc.gpsimd.dma_start(
                            wsg[:],
                            w[ot * P : (ot + 1) * P, kc * W_KC : (kc + 1) * W_KC],
                        )
                        # sign(w) = (w16 & 0x8000) ^ bits(1.0f16)  -> +-1.0
                        nc.vector.tensor_scalar(
                            wsg[:].bitcast(u16),
                            wsg[:].bitcast(u16),
                            0x8000,
                            0x3C00,
                            mybir.AluOpType.bitwise_and,
                            mybir.AluOpType.bitwise_xor,
                        )
                        for g in range(W_KC // P // 4):  # groups of 4
                            tp = tpps.tile([P, 512], f16, tag="tp")
                            for j in range(4):
                                ki = g * 4 + j
                                nc.tensor.transpose(
                                    tp[:, j * P : (j + 1) * P],
                                    wsg[:, ki * P : (ki + 1) * P],
                                    ident[:],
                                )
                            k0 = g * 4
                            src = tp[:].rearrange("p (a b) -> p a b", a=4)
                            if kc == 0:
                                dst = B16[:, k0 : k0 + 4, ot * P : (ot + 1) * P]
                            else:
                                dst = B8[:, k0 : k0 + 4, ot * P : (ot + 1) * P]
                            # alternate copy engine to balance ACT/DVE
                            if (g + oi + copy_flip) % 2 == 0:
                                nc.vector.tensor_copy(dst, src)
                            else:
                                nc.scalar.activation(dst, src, AF.Copy)

            def make_xT(mt):
                xc = xpool.tile([P, IN_F], f16, tag="xc")
                nc.gpsimd.dma_start(xc[:], x[mt * P : (mt + 1) * P, :])
                xT = xtpool.tile([P, KT16, P], f16, tag="xT")
                xT8 = xtpool.tile([P, KT8, P], fp8, tag="xT8")
                for g in range(KT // 4):  # 8 groups of 4 transposes
                    tp = tpps.tile([P, 512], f16, tag="tp")
                    for j in range(4):
                        ki = g * 4 + j
                        nc.tensor.transpose(
                            tp[:, j * P : (j + 1) * P],
                            xc[:, ki * P : (ki + 1) * P],
                            ident[:],
                        )
                    src = tp[:].rearrange("p (a b) -> p a b", a=4)
                    if g < KT16 // 4:
                        nc.scalar.activation(xT[:, g * 4 : g * 4 + 4, :], src, AF.Copy)
                    else:
                        g8 = g - KT16 // 4
                        nc.scalar.activation(
                            xT8[:, g8 * 4 : g8 * 4 + 4, :], src, AF.Copy
                        )
                return xT, xT8

            def mm_band(mt, band, xT, xT8):
                """Full accumulation for one (token tile, band): f16 then DR."""
                ps = mmps.tile([P, NB], f32, tag="ps")
                n0 = band * NB
                for k in range(KT16):
                    nc.tensor.matmul(
                        ps[:], xT[:, k, :], B16[:, k, n0 : n0 + NB],
                        start=(k == 0), stop=False,
                    )
                for c in range(KT8 // 2):
                    nc.tensor.matmul(
                        ps[:],
                        xT8[:, 2 * c : 2 * c + 2, :],
                        B8[:, 2 * c : 2 * c + 2, n0 : n0 + NB],
                        start=False, stop=(c == KT8 // 2 - 1),
                        perf_mode=DR,
                    )
                yq = ypool.tile([P, NB], f32, tag="yq")
                nc.vector.tensor_tensor(
                    yq[:], ps[:], SCB[:, n0 : n0 + NB], mybir.AluOpType.mult
                )
                nc.sync.dma_start(y[mt * P : (mt + 1) * P, n0 : n0 + NB], yq[:])

            # W prep band 0, warm xT tiles, then band-major warm MMs with
            # each next band's prep emitted BEFORE the current band's MMs
            prep_w_band(0)
            warm_xT = [make_xT(mt) for mt in range(WARM)]
            for band in range(NBAND):
                if band + 1 < NBAND:
                    prep_w_band(band + 1)
                for mt in range(WARM):
                    mm_band(mt, band, *warm_xT[mt])

            # steady phase
            for mt in range(WARM, MT):
                xT, xT8 = make_xT(mt)
                for band in range(NBAND):
                    mm_band(mt, band, xT, xT8)

    nc.finalize()
    return nc


def _get_nc():
    if "nc" not in _CACHE:
        _CACHE["nc"] = build()
    return _CACHE["nc"]


def kernel(x, weight, scale):
    global last_result
    nc = _get_nc()
    x = np.ascontiguousarray(np.asarray(x, dtype=np.float32))
    weight = np.ascontiguousarray(np.asarray(weight, dtype=np.float32))
    scale = np.ascontiguousarray(np.asarray(scale, dtype=np.float32))
    in_maps = [
        {
            "x": x,
            "weight": np.ascontiguousarray(weight[c * O_SH : (c + 1) * O_SH]),
            "scale": np.ascontiguousarray(scale[c * O_SH : (c + 1) * O_SH]),
        }
        for c in range(NCORES)
    ]
    res = run_bass_kernel_spmd(nc, in_maps, list(range(NCORES)))
    last_result = res
    return np.concatenate([res.results[c]["y"] for c in range(NCORES)], axis=1)


if __name__ == "__main__":
    rng = np.random.default_rng(0)
    xv = rng.standard_normal((TOKENS, IN_F), dtype=np.float32)
    wv = rng.standard_normal((OUT_F, IN_F), dtype=np.float32)
    sv = np.ones(OUT_F, dtype=np.float32)
    yv = kernel(xv, wv, sv)
    print("out shape:", yv.shape, yv.dtype)
